# revision 8
# baseline (speedup 1.0000x reference)
"""Trainium2 Bass kernel for nn_Aggregator (ragged_sequence overlap-add merge).

Problem geometry (hardcoded — uniform chunking, full masks):
  BATCH=16 docs, C=16 chunks/doc, chunk len L=512, stride 128, D=512, Q=64.
  Merged doc length = 511 + 15*127 = 2416.
  new_x_q[b] = mean over chunks of x_q            [16, 64, 512]
  new_x_d[b] = overlap-add of chunk[:, :511] at offsets c*127, / counts
                                                   [16, 2416, 512]

Sharding: data-parallel over docs — 2 docs per core on 8 NeuronCores.

Kernel layout: output positions p = 127*k + r (k in 0..18, r in 0..126, plus a
3-row tail block k=19). Chunk c's rows split as l = 127*j + r (j in 0..3 full
127-row sub-blocks + 3 tail rows 508..510). Row (c, l) lands at p = 127*(c+j)+r,
so output block k is the sum of sub-block j of chunk k-j (all partition-aligned
at base 0) plus the 3-row tail of chunk k-4. The per-position 1/count scale is
fused into the PSUM-free ScalarE copy (activation Copy with per-partition
scale), so VectorE only does the aligned adds.
"""

import numpy as np

import concourse.bacc as bacc
import concourse.bass as bass
import concourse.mybir as mybir
import concourse.tile as tile
from concourse.bass_utils import run_bass_kernel_spmd

# ---- geometry (hardcoded, matches reference.py) ----
B = 16            # documents
C = 16            # chunks per document
L = 512           # chunk length (tokens)
D = 512           # feature dim
Q = 64            # question tokens
STEP = 127        # tokens each chunk advances (DOC_STRIDE - 1)
DOC_LEN = 2416    # (L-1) + (C-1)*STEP
NBLK = 19         # full 127-row output blocks (19*127 = 2413)
N_CORES = 8
DOCS_PER_CORE = B // N_CORES  # 2
F32 = mybir.dt.float32


def _make_recip() -> np.ndarray:
    """Per-position reciprocal contribution counts, shaped [127, 19] f32
    (column k = positions 127k .. 127k+126). The 3-row tail block (positions
    2413..2415) has count 1 and is handled by a direct DMA copy."""
    denom = np.zeros(DOC_LEN, np.float64)
    for c in range(C):
        denom[c * STEP : c * STEP + (L - 1)] += 1.0
    assert np.all(denom[NBLK * STEP :] == 1.0)
    recip = (1.0 / denom[: NBLK * STEP]).astype(np.float32)
    return np.ascontiguousarray(recip.reshape(NBLK, STEP).T)  # [127, 19]


def _build_program():
    nc = bacc.Bacc("TRN2", target_bir_lowering=False, debug=False)

    xq = nc.dram_tensor("xq", [DOCS_PER_CORE * C, Q, D], F32, kind="ExternalInput").ap()
    xd = nc.dram_tensor("xd", [DOCS_PER_CORE * C, L, D], F32, kind="ExternalInput").ap()
    rc = nc.dram_tensor("recip", [STEP, NBLK], F32, kind="ExternalInput").ap()
    oq = nc.dram_tensor("oq", [DOCS_PER_CORE, Q, D], F32, kind="ExternalOutput").ap()
    od = nc.dram_tensor("od", [DOCS_PER_CORE, DOC_LEN, D], F32, kind="ExternalOutput").ap()

    COPY = mybir.ActivationFunctionType.Copy

    with tile.TileContext(nc) as tc:
        with (
            tc.tile_pool(name="rpool", bufs=1) as rpool,
            tc.tile_pool(name="cpool", bufs=8) as cpool,
            tc.tile_pool(name="tpool", bufs=7) as tpool,
            tc.tile_pool(name="gpool", bufs=3) as gpool,
            tc.tile_pool(name="qpool", bufs=3) as qpool,
            tc.tile_pool(name="qopool", bufs=2) as qopool,
        ):
            rtile = rpool.tile([STEP, NBLK], F32)
            nc.sync.dma_start(out=rtile[:, :], in_=rc)

            for b in range(DOCS_PER_CORE):
                xq_b = xq[b * C : (b + 1) * C]      # [16, 64, 512]
                xd_b = xd[b * C : (b + 1) * C]      # [16, 512, 512]

                # ---- question mean: pairwise tree over the 16 chunk copies ----
                qacc = qpool.tile([Q, 8 * D], F32)
                nc.sync.dma_start(
                    out=qacc[:, :].rearrange("q (c f) -> q c f", f=D),
                    in_=xq_b[0:8].rearrange("c q f -> q c f"),
                )
                qnext = qpool.tile([Q, 8 * D], F32)
                nc.sync.dma_start(
                    out=qnext[:, :].rearrange("q (c f) -> q c f", f=D),
                    in_=xq_b[8:16].rearrange("c q f -> q c f"),
                )
                nc.vector.tensor_add(qacc[:, :], qacc[:, :], qnext[:, :])
                w = 4 * D
                while w >= D:
                    nc.vector.tensor_add(qacc[:, 0:w], qacc[:, 0:w], qacc[:, w : 2 * w])
                    w //= 2
                qo = qopool.tile([Q, D], F32)
                nc.scalar.activation(qo[:, :], qacc[:, 0:D], COPY, scale=1.0 / C)
                nc.sync.dma_start(out=oq[b], in_=qo[:, :])

                # ---- document overlap-add ----
                chunk_tiles: dict[int, object] = {}
                tail_tiles: dict[int, object] = {}
                group_tiles: dict[int, object] = {}

                def emit_block(k: int):
                    """Emit ops computing output block k into its group tile."""
                    g, slot = divmod(k, 4)
                    if slot == 0:
                        group_tiles[g] = gpool.tile([STEP, 4 * D], F32, name=f"grp_{b}_{g}", tag="grp")
                    gt = group_tiles[g]
                    gs = gt[:, slot * D : (slot + 1) * D]
                    # full 127-row contributors: sub-block j of chunk k-j
                    terms = [
                        chunk_tiles[k - j][:, j * D : (j + 1) * D]
                        for j in range(4)
                        if 0 <= k - j < C
                    ]
                    tail = tail_tiles.get(k - 4)  # 3-row tail of chunk k-4
                    scale = rtile[:, k : k + 1]
                    if len(terms) == 1 and tail is None:         # k == 0
                        nc.scalar.activation(gs, terms[0], COPY, scale=scale)
                        return
                    if len(terms) == 1:                          # k == 18
                        nc.vector.tensor_copy(gs, terms[0])
                        nc.vector.tensor_add(gs[0:3, :], gs[0:3, :], tail[:, :])
                        nc.scalar.activation(gs, gs, COPY, scale=scale)
                        return
                    nc.vector.tensor_add(gs, terms[0], terms[1])
                    for t in terms[2:]:
                        nc.vector.tensor_add(gs, gs, t)
                    if tail is not None:
                        nc.vector.tensor_add(gs[0:3, :], gs[0:3, :], tail[:, :])
                    nc.scalar.activation(gs, gs, COPY, scale=scale)

                def emit_group_out(k: int):
                    """DMA a finished 4-block group (or the last 3-block one)."""
                    g, slot = divmod(k, 4)
                    nblk_in_g = slot + 1
                    gt = group_tiles[g]
                    rows = nblk_in_g * STEP
                    nc.sync.dma_start(
                        out=od[b, g * 4 * STEP : g * 4 * STEP + rows, :].rearrange(
                            "(k p) f -> p k f", p=STEP
                        ),
                        in_=gt[:, 0 : nblk_in_g * D].rearrange("p (k f) -> p k f", f=D),
                    )

                for c in range(C):
                    ct = cpool.tile([STEP, 4 * D], F32, name=f"chunk_{b}_{c}", tag="chunk")
                    nc.sync.dma_start(
                        out=ct[:, :].rearrange("p (j f) -> p j f", f=D),
                        in_=xd_b[c][0 : 4 * STEP].rearrange("(j p) f -> p j f", p=STEP),
                    )
                    chunk_tiles[c] = ct
                    if c <= C - 2:
                        tl = tpool.tile([3, D], F32, name=f"tail_{b}_{c}", tag="tail")
                        nc.sync.dma_start(out=tl[:, :], in_=xd_b[c][4 * STEP : L - 1])
                        tail_tiles[c] = tl
                    emit_block(c)
                    if c % 4 == 3:
                        emit_group_out(c)
                for k in range(C, NBLK):
                    emit_block(k)
                emit_group_out(NBLK - 1)
                # tail block (positions 2413..2415): single contributor, count 1
                nc.sync.dma_start(
                    out=od[b, NBLK * STEP : DOC_LEN, :],
                    in_=xd_b[C - 1][4 * STEP : L - 1, :],
                )

    nc.compile()
    return nc


_PROGRAM_CACHE: list = []


def _get_program():
    if not _PROGRAM_CACHE:
        _PROGRAM_CACHE.append(_build_program())
    return _PROGRAM_CACHE[0]


def kernel(x_q, x_d, **_unused) -> tuple[np.ndarray, np.ndarray]:
    x_q = np.ascontiguousarray(np.asarray(x_q), dtype=np.float32)
    x_d = np.ascontiguousarray(np.asarray(x_d), dtype=np.float32)
    assert x_q.shape == (B * C, Q, D) and x_d.shape == (B * C, L, D)

    nc = _get_program()
    recip = _make_recip()
    in_maps = []
    for r in range(N_CORES):
        s = slice(r * DOCS_PER_CORE * C, (r + 1) * DOCS_PER_CORE * C)
        in_maps.append({"xq": x_q[s], "xd": x_d[s], "recip": recip})

    res = run_bass_kernel_spmd(nc, in_maps, core_ids=list(range(N_CORES)))
    new_x_q = np.concatenate([r["oq"] for r in res.results], axis=0)
    new_x_d = np.concatenate([r["od"] for r in res.results], axis=0)
    return new_x_q, new_x_d
